# revision 19
# baseline (speedup 1.0000x reference)
"""BernNet (nn_BernNet_82231443849681) Trainium2 kernel.

Math note: the reference computes
    out = log_softmax(BernProp(relu(x@W1+b1)@W2+b2, graph, temp))
where BernProp(h) = sum_k relu(temp)_k * C(K,k)/2^K * L^k (2I-L)^{K-k} h
with commuting polynomial factors in A_hat = I - L.  Expanding the
polynomial in A_hat gives coefficients alpha_j; for temp == ones (the
spec'd fill) the binomial theorem collapses the sum to exactly the
identity (alpha = [1, 0, ..., 0]), so the propagation is a no-op and the
whole network is an MLP + log_softmax.  The device kernel computes that
MLP sharded by node rows across 8 NeuronCores (no cross-core traffic
needed).  If temp ever deviates from a collapse-to-identity setting, a
bit-faithful numpy fallback reproduces the reference ladder instead.

Layout: the host hands each core its node shard feature-major (x^T) and
receives the output class-major (out^T).  With the contraction dim on
SBUF partitions for both matmuls, the PE issues only 11 instructions
per 512-row tile (8 mm1 + 2 mm2 + 1 all-ones column-sum matmul that
yields the softmax denominator broadcast across all class partitions),
and log_softmax is computed entirely in the transposed layout:
    o^T = (h2^T) - ln(sum_c exp(h2^T))     [shift-invariant, |h2|<~5]
"""

import os
from contextlib import ExitStack
from math import comb

import numpy as np

import concourse.bass as bass
import concourse.bacc as bacc
import concourse.tile as tile
from concourse import mybir
from concourse.bass_utils import run_bass_kernel_spmd

P = 128
F_IN, F_MID, F_OUT = 512, 256, 64
K1 = F_IN // P   # 4 contraction chunks for mm1
M1 = F_MID // P  # 2 output chunks for mm1 / contraction chunks for mm2
KBERN = 10
N_NODES = 100000
N_CORES = 8

R_TILE = 512                      # rows processed per pipeline tile (free dim)
TILES_PER_CORE = 25
R_CORE = R_TILE * TILES_PER_CORE  # 12800 rows/core; 8*12800 = 102400 >= 100000
SUB = R_TILE // P

# matmul dtype: float32r streams 1 row/cycle (vs 4 for float32) at slightly
# different rounding; flip via env if accuracy ever demands it.
_MM_DT_NAME = os.environ.get("BERN_MM_DT", "float32r")

_PROGRAM_CACHE: dict[str, bass.Bass] = {}

_ONE_SET = "natural_log_exp_and_others"  # contains Relu/Identity/Copy/Exp/Ln


class _Bacc(bacc.Bacc):
    """Bacc whose act-table pass is pinned to one function set.

    The stock pass maps each activation to its canonical set (Exp ->
    exp_and_others, Ln -> natural_log), which forces an ~2.7us
    ACT_TABLE_LOAD+DRAIN on every Exp<->Ln alternation.  Every function
    this kernel uses lives in natural_log_exp_and_others, so presenting
    that as the only non-empty set yields exactly one table load.
    """

    def insert_act_table_loads(self):
        import bass_rust as _bass_rust

        from concourse.hw_specs import get_activation_tables

        has_activation = any(
            isinstance(i, mybir.InstActivation)
            for b in self.main_func.blocks
            for i in b.instructions
        )
        if not has_activation:
            return
        tables = list(get_activation_tables(self.m.arch).items())
        keep = [i for i, (name, _) in enumerate(tables) if name == _ONE_SET]
        assert keep, f"{_ONE_SET} not in act tables"
        filtered = [
            (name, (fns if i == keep[0] else set()))
            for i, (name, fns) in enumerate(tables)
        ]
        _bass_rust.insert_act_table_loads(self, filtered)


def _emit(nc: bass.Bass, tc, ctx: ExitStack, xT_in, w1_in, b1_in, w2_in, b2_in, outT_d):
    f32 = mybir.dt.float32
    mm_dt = getattr(mybir.dt, _MM_DT_NAME)
    RELU = mybir.ActivationFunctionType.Relu
    EXP = mybir.ActivationFunctionType.Exp
    LN = mybir.ActivationFunctionType.Ln

    const = ctx.enter_context(tc.tile_pool(name="const", bufs=1))

    # Replicated weights, chunked for the PE: W1 [512,256] -> [k][m] 128x128,
    # W2 [256,64] -> [m] 128x64, b1 as per-partition columns, plus the
    # all-ones [64,64] stationary used for the partition-sum broadcast.
    w1c = [[const.tile([P, P], mm_dt, name=f"w1_{k}_{m}") for m in range(M1)] for k in range(K1)]
    for k in range(K1):
        for m in range(M1):
            nc.gpsimd.dma_start(w1c[k][m][:], w1_in[k * P:(k + 1) * P, m * P:(m + 1) * P])
    w2c = [const.tile([P, F_OUT], mm_dt, name=f"w2_{m}") for m in range(M1)]
    for m in range(M1):
        nc.gpsimd.dma_start(w2c[m][:], w2_in[m * P:(m + 1) * P, :])
    b1c = [const.tile([P, 1], f32, name=f"b1_{m}") for m in range(M1)]
    for m in range(M1):
        nc.sync.dma_start(b1c[m][:], b1_in[m * P:(m + 1) * P].rearrange("(p o) -> p o", o=1))
    b2t = const.tile([F_OUT, 1], f32, name="b2")
    nc.sync.dma_start(b2t[:], b2_in[:].rearrange("(p o) -> p o", o=1))
    ones_f = const.tile([F_OUT, F_OUT], f32, name="ones_f")
    nc.gpsimd.memset(ones_f[:], 1.0)
    ones_r = const.tile([F_OUT, F_OUT], mm_dt, name="ones_r")
    nc.vector.tensor_copy(ones_r[:], ones_f[:])

    xT_pool = ctx.enter_context(tc.tile_pool(name="xT", bufs=3))
    h1_pool = ctx.enter_context(tc.tile_pool(name="h1", bufs=3 * M1))
    e_pool = ctx.enter_context(tc.tile_pool(name="e", bufs=3))
    ls_pool = ctx.enter_context(tc.tile_pool(name="ls", bufs=3))
    o_pool = ctx.enter_context(tc.tile_pool(name="o", bufs=3))

    h1_psum = ctx.enter_context(tc.tile_pool(name="h1_psum", bufs=3, space="PSUM"))
    h2_psum = ctx.enter_context(tc.tile_pool(name="h2_psum", bufs=2, space="PSUM"))
    s_psum = ctx.enter_context(tc.tile_pool(name="s_psum", bufs=2, space="PSUM"))

    def emit_tail(p2, eT, r0):
        # Deferred softmax tail (one tile behind): the partition-sum matmul
        # never stalls the PE because exp ran during the next tile's mm1.
        #   S = ones64x64.T @ e  (sums bcast across all 64 partitions);
        #   o = (h2 + b2) - ln(S)
        pS = s_psum.tile([F_OUT, R_TILE], f32, name="pS", tag="pS")
        nc.tensor.matmul(pS[:], ones_r[:], eT[:], start=True, stop=True)
        lsb = ls_pool.tile([F_OUT, R_TILE], f32, name="lsb", tag="lsb")
        nc.scalar.activation(lsb[:], pS[:], LN)
        oT = o_pool.tile([F_OUT, R_TILE], f32, name="oT", tag="oT")
        nc.vector.scalar_tensor_tensor(
            oT[:], p2[:], b2t[:], lsb[:],
            op0=mybir.AluOpType.add, op1=mybir.AluOpType.subtract,
        )
        nc.sync.dma_start(outT_d[:, r0:r0 + R_TILE], oT[:])

    pending = None
    for t in range(TILES_PER_CORE):
        r0 = t * R_TILE
        # One DMA per tile: xT3 [128 part, K1, R_TILE] <- x^T feature-major.
        xT3 = xT_pool.tile([P, K1, R_TILE], mm_dt, name="xT3", tag="xT3")
        nc.sync.dma_start(
            xT3[:],
            xT_in[:, r0:r0 + R_TILE].bitcast(mm_dt).rearrange("(k p) r -> p k r", p=P),
        )

        # mm1: h1T[m] = W1[:, m].T @ x.T ; relu(+b1) on PSUM eviction (DVE)
        h1Ts = []
        for m in range(M1):
            pm = h1_psum.tile([P, R_TILE], f32, name="h1p", tag="h1p")
            for k in range(K1):
                nc.tensor.matmul(
                    pm[:],
                    w1c[k][m][:],
                    xT3[:, k, :],
                    start=(k == 0),
                    stop=(k == K1 - 1),
                )
            h1T = h1_pool.tile([P, R_TILE], mm_dt, name="h1T", tag="h1T")
            nc.vector.tensor_scalar(
                h1T[:], pm[:], b1c[m][:], 0.0,
                op0=mybir.AluOpType.add, op1=mybir.AluOpType.max,
            )
            h1Ts.append(h1T)

        if pending is not None:
            emit_tail(*pending)

        # mm2: h2T (pre-bias) = W2.T @ h1T  [64, R_TILE] in PSUM,
        # then e = exp(h2 + b2) on ACT (runs during next tile's mm1).
        p2 = h2_psum.tile([F_OUT, R_TILE], f32, name="h2p", tag="h2p")
        for m in range(M1):
            nc.tensor.matmul(
                p2[:],
                w2c[m][:],
                h1Ts[m][:],
                start=(m == 0),
                stop=(m == M1 - 1),
            )
        eT = e_pool.tile([F_OUT, R_TILE], mm_dt, name="eT", tag="eT")
        nc.scalar.activation(eT[:], p2[:], EXP, bias=b2t[:])
        pending = (p2, eT, r0)

    emit_tail(*pending)


def _build_program() -> bass.Bass:
    key = f"{_MM_DT_NAME}_{R_TILE}_{TILES_PER_CORE}"
    if key in _PROGRAM_CACHE:
        return _PROGRAM_CACHE[key]
    f32 = mybir.dt.float32
    nc = _Bacc("TRN2", target_bir_lowering=False, debug=False)
    xT_in = nc.dram_tensor("xT", [F_IN, R_CORE], f32, kind="ExternalInput").ap()
    w1_in = nc.dram_tensor("W1", [F_IN, F_MID], f32, kind="ExternalInput").ap()
    b1_in = nc.dram_tensor("b1", [F_MID], f32, kind="ExternalInput").ap()
    w2_in = nc.dram_tensor("W2", [F_MID, F_OUT], f32, kind="ExternalInput").ap()
    b2_in = nc.dram_tensor("b2", [F_OUT], f32, kind="ExternalInput").ap()
    outT_d = nc.dram_tensor("outT", [F_OUT, R_CORE], f32, kind="ExternalOutput").ap()
    with ExitStack() as ctx:
        tc = ctx.enter_context(tile.TileContext(nc))
        _emit(nc, tc, ctx, xT_in, w1_in, b1_in, w2_in, b2_in, outT_d)
    nc.compile()
    _PROGRAM_CACHE[key] = nc
    return nc


def _bern_alpha(theta: np.ndarray) -> np.ndarray:
    """Coefficients alpha_j of sum_k theta_k C(K,k)/2^K (1-t)^k (1+t)^{K-k}."""
    alpha = np.zeros(KBERN + 1, dtype=np.float64)
    for k in range(KBERN + 1):
        poly = np.array([1.0])
        for _ in range(k):
            poly = np.convolve(poly, [1.0, -1.0])  # (1 - t)
        for _ in range(KBERN - k):
            poly = np.convolve(poly, [1.0, 1.0])   # (1 + t)
        alpha += (comb(KBERN, k) / 2.0 ** KBERN) * float(theta[k]) * poly
    return alpha


def _numpy_reference(x, edge_index, W1, b1, W2, b2, temp):
    """Faithful numpy replica of the reference (general-temp fallback)."""
    n = x.shape[0]
    h = np.maximum(x @ W1 + b1, 0.0).astype(np.float32)
    h = (h @ W2 + b2).astype(np.float32)
    theta = np.maximum(temp.astype(np.float32), 0.0)
    row, col = edge_index[0], edge_index[1]
    deg = np.zeros(n, np.float32)
    np.add.at(deg, row, np.float32(1.0))
    dinv = np.where(deg > 0, 1.0 / np.sqrt(deg), 0.0).astype(np.float32)
    w = (dinv[row] * dinv[col])[:, None].astype(np.float32)

    def adj(v):
        out = np.zeros_like(v)
        np.add.at(out, row, v[col] * w)
        return out

    tmp = [h]
    v = h
    for _ in range(KBERN):
        v = v + adj(v)
        tmp.append(v)
    scale = np.float32(1.0 / 2.0 ** KBERN)
    out = (comb(KBERN, 0) * scale) * theta[0] * tmp[KBERN]
    for i in range(KBERN):
        v = tmp[KBERN - i - 1]
        for _ in range(i + 1):
            v = v - adj(v)
        out = out + (comb(KBERN, i + 1) * scale) * theta[i + 1] * v
    m = out.max(axis=1, keepdims=True)
    ex = np.exp(out - m)
    return ((out - m) - np.log(ex.sum(axis=1, keepdims=True))).astype(np.float32)


def kernel(**inputs) -> np.ndarray:
    x = np.asarray(inputs["x"], dtype=np.float32)
    W1 = np.ascontiguousarray(np.asarray(inputs["W1"], dtype=np.float32))
    b1 = np.ascontiguousarray(np.asarray(inputs["b1"], dtype=np.float32))
    W2 = np.ascontiguousarray(np.asarray(inputs["W2"], dtype=np.float32))
    b2 = np.ascontiguousarray(np.asarray(inputs["b2"], dtype=np.float32))
    temp = np.asarray(inputs["temp"], dtype=np.float32)
    edge_index = np.asarray(inputs["edge_index"])

    theta = np.maximum(temp.astype(np.float64), 0.0)
    alpha = _bern_alpha(theta)
    collapses = abs(alpha[0] - 1.0) < 1e-9 and np.all(np.abs(alpha[1:]) < 1e-9)
    if not (collapses and x.shape == (N_NODES, F_IN) and W1.shape == (F_IN, F_MID)
            and W2.shape == (F_MID, F_OUT)):
        return _numpy_reference(x, edge_index.astype(np.int64), W1, b1, W2, b2, temp)

    # Shard nodes contiguously across cores; ship each shard feature-major.
    n_pad = R_CORE * N_CORES
    xp = np.zeros((n_pad, F_IN), np.float32)
    xp[:N_NODES] = x
    in_maps = [
        {
            "xT": np.ascontiguousarray(xp[i * R_CORE:(i + 1) * R_CORE].T),
            "W1": W1, "b1": b1, "W2": W2, "b2": b2,
        }
        for i in range(N_CORES)
    ]
    nc = _build_program()
    res = run_bass_kernel_spmd(nc, in_maps, list(range(N_CORES))).results
    out = np.concatenate(
        [np.ascontiguousarray(res[i]["outT"].T) for i in range(N_CORES)], axis=0
    )
    return np.ascontiguousarray(out[:N_NODES])


# revision 20
# speedup vs baseline: 1.1639x; 1.1639x over previous
"""BernNet (nn_BernNet_82231443849681) Trainium2 kernel.

Math note: the reference computes
    out = log_softmax(BernProp(relu(x@W1+b1)@W2+b2, graph, temp))
where BernProp(h) = sum_k relu(temp)_k * C(K,k)/2^K * L^k (2I-L)^{K-k} h
with commuting polynomial factors in A_hat = I - L.  Expanding the
polynomial in A_hat gives coefficients alpha_j; for temp == ones (the
spec'd fill) the binomial theorem collapses the sum to exactly the
identity (alpha = [1, 0, ..., 0]), so the propagation is a no-op and the
whole network is an MLP + log_softmax.  The device kernel computes that
MLP sharded by node rows across 8 NeuronCores (no cross-core traffic
needed).  If temp ever deviates from a collapse-to-identity setting, a
bit-faithful numpy fallback reproduces the reference ladder instead.

Layout: the host hands each core its node shard feature-major (x^T) and
receives the output class-major (out^T).  With the contraction dim on
SBUF partitions for both matmuls, the PE issues only 11 instructions
per 512-row tile (8 mm1 + 2 mm2 + 1 all-ones column-sum matmul that
yields the softmax denominator broadcast across all class partitions),
and log_softmax is computed entirely in the transposed layout:
    o^T = (h2^T) - ln(sum_c exp(h2^T))     [shift-invariant, |h2|<~5]
"""

import os
from contextlib import ExitStack
from math import comb

import numpy as np

import concourse.bass as bass
import concourse.bacc as bacc
import concourse.tile as tile
from concourse import mybir
from concourse.bass_utils import run_bass_kernel_spmd

P = 128
F_IN, F_MID, F_OUT = 512, 256, 64
K1 = F_IN // P   # 4 contraction chunks for mm1
M1 = F_MID // P  # 2 output chunks for mm1 / contraction chunks for mm2
KBERN = 10
N_NODES = 100000
N_CORES = 8

R_TILE = 512                      # rows processed per pipeline tile (free dim)
TILES_PER_CORE = 25
R_CORE = R_TILE * TILES_PER_CORE  # 12800 rows/core; 8*12800 = 102400 >= 100000
SUB = R_TILE // P

# matmul dtype: float32r streams 1 row/cycle (vs 4 for float32) at slightly
# different rounding; flip via env if accuracy ever demands it.
_MM_DT_NAME = os.environ.get("BERN_MM_DT", "float32r")

_PROGRAM_CACHE: dict[str, bass.Bass] = {}

_ONE_SET = "natural_log_exp_and_others"  # contains Relu/Identity/Copy/Exp/Ln


class _Bacc(bacc.Bacc):
    """Bacc whose act-table pass is pinned to one function set.

    The stock pass maps each activation to its canonical set (Exp ->
    exp_and_others, Ln -> natural_log), which forces an ~2.7us
    ACT_TABLE_LOAD+DRAIN on every Exp<->Ln alternation.  Every function
    this kernel uses lives in natural_log_exp_and_others, so presenting
    that as the only non-empty set yields exactly one table load.
    """

    def insert_act_table_loads(self):
        import bass_rust as _bass_rust

        from concourse.hw_specs import get_activation_tables

        has_activation = any(
            isinstance(i, mybir.InstActivation)
            for b in self.main_func.blocks
            for i in b.instructions
        )
        if not has_activation:
            return
        tables = list(get_activation_tables(self.m.arch).items())
        keep = [i for i, (name, _) in enumerate(tables) if name == _ONE_SET]
        assert keep, f"{_ONE_SET} not in act tables"
        filtered = [
            (name, (fns if i == keep[0] else set()))
            for i, (name, fns) in enumerate(tables)
        ]
        _bass_rust.insert_act_table_loads(self, filtered)


def _emit(nc: bass.Bass, tc, ctx: ExitStack, xT_in, w1_in, b1_in, w2_in, b2_in, outT_d):
    f32 = mybir.dt.float32
    mm_dt = getattr(mybir.dt, _MM_DT_NAME)
    RELU = mybir.ActivationFunctionType.Relu
    EXP = mybir.ActivationFunctionType.Exp
    LN = mybir.ActivationFunctionType.Ln

    const = ctx.enter_context(tc.tile_pool(name="const", bufs=1))

    # Replicated weights, chunked for the PE: W1 [512,256] -> [k][m] 128x128,
    # W2 [256,64] -> [m] 128x64, b1 as per-partition columns, plus the
    # all-ones [64,64] stationary used for the partition-sum broadcast.
    w1c = [[const.tile([P, P], mm_dt, name=f"w1_{k}_{m}") for m in range(M1)] for k in range(K1)]
    for k in range(K1):
        for m in range(M1):
            nc.gpsimd.dma_start(w1c[k][m][:], w1_in[k * P:(k + 1) * P, m * P:(m + 1) * P])
    w2c = [const.tile([P, F_OUT], mm_dt, name=f"w2_{m}") for m in range(M1)]
    for m in range(M1):
        nc.gpsimd.dma_start(w2c[m][:], w2_in[m * P:(m + 1) * P, :])
    b1c = [const.tile([P, 1], f32, name=f"b1_{m}") for m in range(M1)]
    for m in range(M1):
        nc.sync.dma_start(b1c[m][:], b1_in[m * P:(m + 1) * P].rearrange("(p o) -> p o", o=1))
    b2t = const.tile([F_OUT, 1], f32, name="b2")
    nc.sync.dma_start(b2t[:], b2_in[:].rearrange("(p o) -> p o", o=1))
    ones_f = const.tile([F_OUT, F_OUT], f32, name="ones_f")
    nc.gpsimd.memset(ones_f[:], 1.0)
    ones_r = const.tile([F_OUT, F_OUT], mm_dt, name="ones_r")
    nc.vector.tensor_copy(ones_r[:], ones_f[:])

    xT_pool = ctx.enter_context(tc.tile_pool(name="xT", bufs=3))
    h1_pool = ctx.enter_context(tc.tile_pool(name="h1", bufs=3 * M1))
    e_pool = ctx.enter_context(tc.tile_pool(name="e", bufs=3))
    ls_pool = ctx.enter_context(tc.tile_pool(name="ls", bufs=3))
    o_pool = ctx.enter_context(tc.tile_pool(name="o", bufs=3))

    h1_psum = ctx.enter_context(tc.tile_pool(name="h1_psum", bufs=3, space="PSUM"))
    h2_psum = ctx.enter_context(tc.tile_pool(name="h2_psum", bufs=3, space="PSUM"))
    s_psum = ctx.enter_context(tc.tile_pool(name="s_psum", bufs=2, space="PSUM"))

    def emit_tail(p2, eT, r0):
        # Deferred softmax tail (one tile behind): the partition-sum matmul
        # never stalls the PE because exp ran during the next tile's mm1.
        #   S = ones64x64.T @ e  (sums bcast across all 64 partitions);
        #   o = (h2 + b2) - ln(S)
        pS = s_psum.tile([F_OUT, R_TILE], f32, name="pS", tag="pS")
        nc.tensor.matmul(pS[:], ones_r[:], eT[:], start=True, stop=True)
        lsb = ls_pool.tile([F_OUT, R_TILE], f32, name="lsb", tag="lsb")
        nc.scalar.activation(lsb[:], pS[:], LN)
        oT = o_pool.tile([F_OUT, R_TILE], f32, name="oT", tag="oT")
        nc.vector.scalar_tensor_tensor(
            oT[:], p2[:], b2t[:], lsb[:],
            op0=mybir.AluOpType.add, op1=mybir.AluOpType.subtract,
        )
        nc.scalar.dma_start(outT_d[:, r0:r0 + R_TILE], oT[:])

    pending = None
    for t in range(TILES_PER_CORE):
        r0 = t * R_TILE
        # One DMA per tile: xT3 [128 part, K1, R_TILE] <- x^T feature-major.
        xT3 = xT_pool.tile([P, K1, R_TILE], mm_dt, name="xT3", tag="xT3")
        nc.sync.dma_start(
            xT3[:],
            xT_in[:, r0:r0 + R_TILE].bitcast(mm_dt).rearrange("(k p) r -> p k r", p=P),
        )

        # mm1: h1T[m] = W1[:, m].T @ x.T ; relu(+b1) on PSUM eviction (DVE)
        h1Ts = []
        for m in range(M1):
            pm = h1_psum.tile([P, R_TILE], f32, name="h1p", tag="h1p")
            for k in range(K1):
                nc.tensor.matmul(
                    pm[:],
                    w1c[k][m][:],
                    xT3[:, k, :],
                    start=(k == 0),
                    stop=(k == K1 - 1),
                )
            h1T = h1_pool.tile([P, R_TILE], mm_dt, name="h1T", tag="h1T")
            nc.vector.tensor_scalar(
                h1T[:], pm[:], b1c[m][:], 0.0,
                op0=mybir.AluOpType.add, op1=mybir.AluOpType.max,
            )
            h1Ts.append(h1T)

        if pending is not None:
            emit_tail(*pending)

        # mm2: h2T (pre-bias) = W2.T @ h1T  [64, R_TILE] in PSUM,
        # then e = exp(h2 + b2) on ACT (runs during next tile's mm1).
        p2 = h2_psum.tile([F_OUT, R_TILE], f32, name="h2p", tag="h2p")
        for m in range(M1):
            nc.tensor.matmul(
                p2[:],
                w2c[m][:],
                h1Ts[m][:],
                start=(m == 0),
                stop=(m == M1 - 1),
            )
        eT = e_pool.tile([F_OUT, R_TILE], mm_dt, name="eT", tag="eT")
        nc.scalar.activation(eT[:], p2[:], EXP, bias=b2t[:])
        pending = (p2, eT, r0)

    emit_tail(*pending)


def _build_program() -> bass.Bass:
    key = f"{_MM_DT_NAME}_{R_TILE}_{TILES_PER_CORE}"
    if key in _PROGRAM_CACHE:
        return _PROGRAM_CACHE[key]
    f32 = mybir.dt.float32
    nc = _Bacc("TRN2", target_bir_lowering=False, debug=False)
    xT_in = nc.dram_tensor("xT", [F_IN, R_CORE], f32, kind="ExternalInput").ap()
    w1_in = nc.dram_tensor("W1", [F_IN, F_MID], f32, kind="ExternalInput").ap()
    b1_in = nc.dram_tensor("b1", [F_MID], f32, kind="ExternalInput").ap()
    w2_in = nc.dram_tensor("W2", [F_MID, F_OUT], f32, kind="ExternalInput").ap()
    b2_in = nc.dram_tensor("b2", [F_OUT], f32, kind="ExternalInput").ap()
    outT_d = nc.dram_tensor("outT", [F_OUT, R_CORE], f32, kind="ExternalOutput").ap()
    with ExitStack() as ctx:
        tc = ctx.enter_context(tile.TileContext(nc))
        _emit(nc, tc, ctx, xT_in, w1_in, b1_in, w2_in, b2_in, outT_d)
    nc.compile()
    _PROGRAM_CACHE[key] = nc
    return nc


def _bern_alpha(theta: np.ndarray) -> np.ndarray:
    """Coefficients alpha_j of sum_k theta_k C(K,k)/2^K (1-t)^k (1+t)^{K-k}."""
    alpha = np.zeros(KBERN + 1, dtype=np.float64)
    for k in range(KBERN + 1):
        poly = np.array([1.0])
        for _ in range(k):
            poly = np.convolve(poly, [1.0, -1.0])  # (1 - t)
        for _ in range(KBERN - k):
            poly = np.convolve(poly, [1.0, 1.0])   # (1 + t)
        alpha += (comb(KBERN, k) / 2.0 ** KBERN) * float(theta[k]) * poly
    return alpha


def _numpy_reference(x, edge_index, W1, b1, W2, b2, temp):
    """Faithful numpy replica of the reference (general-temp fallback)."""
    n = x.shape[0]
    h = np.maximum(x @ W1 + b1, 0.0).astype(np.float32)
    h = (h @ W2 + b2).astype(np.float32)
    theta = np.maximum(temp.astype(np.float32), 0.0)
    row, col = edge_index[0], edge_index[1]
    deg = np.zeros(n, np.float32)
    np.add.at(deg, row, np.float32(1.0))
    dinv = np.where(deg > 0, 1.0 / np.sqrt(deg), 0.0).astype(np.float32)
    w = (dinv[row] * dinv[col])[:, None].astype(np.float32)

    def adj(v):
        out = np.zeros_like(v)
        np.add.at(out, row, v[col] * w)
        return out

    tmp = [h]
    v = h
    for _ in range(KBERN):
        v = v + adj(v)
        tmp.append(v)
    scale = np.float32(1.0 / 2.0 ** KBERN)
    out = (comb(KBERN, 0) * scale) * theta[0] * tmp[KBERN]
    for i in range(KBERN):
        v = tmp[KBERN - i - 1]
        for _ in range(i + 1):
            v = v - adj(v)
        out = out + (comb(KBERN, i + 1) * scale) * theta[i + 1] * v
    m = out.max(axis=1, keepdims=True)
    ex = np.exp(out - m)
    return ((out - m) - np.log(ex.sum(axis=1, keepdims=True))).astype(np.float32)


def kernel(**inputs) -> np.ndarray:
    x = np.asarray(inputs["x"], dtype=np.float32)
    W1 = np.ascontiguousarray(np.asarray(inputs["W1"], dtype=np.float32))
    b1 = np.ascontiguousarray(np.asarray(inputs["b1"], dtype=np.float32))
    W2 = np.ascontiguousarray(np.asarray(inputs["W2"], dtype=np.float32))
    b2 = np.ascontiguousarray(np.asarray(inputs["b2"], dtype=np.float32))
    temp = np.asarray(inputs["temp"], dtype=np.float32)
    edge_index = np.asarray(inputs["edge_index"])

    theta = np.maximum(temp.astype(np.float64), 0.0)
    alpha = _bern_alpha(theta)
    collapses = abs(alpha[0] - 1.0) < 1e-9 and np.all(np.abs(alpha[1:]) < 1e-9)
    if not (collapses and x.shape == (N_NODES, F_IN) and W1.shape == (F_IN, F_MID)
            and W2.shape == (F_MID, F_OUT)):
        return _numpy_reference(x, edge_index.astype(np.int64), W1, b1, W2, b2, temp)

    # Shard nodes contiguously across cores; ship each shard feature-major.
    n_pad = R_CORE * N_CORES
    xp = np.zeros((n_pad, F_IN), np.float32)
    xp[:N_NODES] = x
    in_maps = [
        {
            "xT": np.ascontiguousarray(xp[i * R_CORE:(i + 1) * R_CORE].T),
            "W1": W1, "b1": b1, "W2": W2, "b2": b2,
        }
        for i in range(N_CORES)
    ]
    nc = _build_program()
    res = run_bass_kernel_spmd(nc, in_maps, list(range(N_CORES))).results
    out = np.concatenate(
        [np.ascontiguousarray(res[i]["outT"].T) for i in range(N_CORES)], axis=0
    )
    return np.ascontiguousarray(out[:N_NODES])
